# revision 33
# baseline (speedup 1.0000x reference)
"""Trainium2 Bass kernel for the DUAN conditioned-normalization problem.

Contract: kernel(**inputs) takes FULL inputs (B=8 samples), shards one sample
per NeuronCore (8 cores), runs a single Bass/Tile kernel SPMD, and gathers the
full [8, 512, 8192] output.

Per-sample math (matches the jax reference):
  mu_c/var_c over L per channel; mu_l/var_l over (C,L);
  g = sigmoid(gw2 @ relu(gw1 @ c + gb1) + gb2); g_mix = mean_L(g)
  gamma,beta = mw2 @ relu(mw1 @ mean_L(c) + mb1) + mb2
  mu = g_mix*mu_c + (1-g_mix)*mu_l ; sigma likewise from sqrt(var+eps)
  y = (1+gamma)*(x-mu)/sigma + beta ; keep top-k channels by mean_L |y|.

V11 layout (x bf16, c fp8-e4m3 with per-channel error-feedback rounding so
mean_L(c8) stays exact; out bf16 -> ~21 MiB HBM/core):
  Phase 1 streams c one slot ahead of x; PE runs both gate layers (fp8
  weights) and accumulates bf16-mw1 @ fp8-c into one PSUM bank (cond-MLP
  pooling for free); ACT runs the sigmoid with g-mean accumulators plus six
  of eight gate relus (table set pre-warmed at t=0); DVE runs bn_stats
  channel stats plus the other two relus.  Finalize fuses cross-partition
  sum+broadcast in one ones-matmul, uses a single 5-wide sqrt (table
  preloaded by a dummy op), and folds everything into per-channel
  A=(1+gamma)/sigma, B=beta-mu*A.  Phase 2 streams UNMASKED y=A*x+B to HBM
  as 2-MiB tiles immediately (out-DMA hides the imp work); imp=sum|y|
  accumulates meanwhile on ACT (Abs+accum from x) and DVE (abs-reduce of the
  streamed tiles); channels are ranked by count against a PE-broadcast fp32
  imp matrix; finally one indirect DMA per group scatters zero rows over the
  dropped channels (kept channels get out-of-bounds indices, skipped via
  bounds_check), fenced behind the y writes by a same-queue read-back.
"""

import sys

sys.path.insert(0, "/opt/trn_rl_repo")

import numpy as np

B = 8
C = 512
L = 8192
H = 128
CG = 4           # channel groups of 128 partitions
SL = 1024        # phase-1 supertile width along L (gate granularity)
NST = L // SL    # 8
LB = 512         # bn_stats / matmul block
NLB = L // LB    # 16
XC = 2048        # phase-1 slot width along L
NXC = L // XC    # 4
YC = 4096        # phase-2 chunk
NYC = L // YC    # 2
KEEP = max(1, int(C * 0.7))  # 358
EPS = 1e-5

W8 = True        # gate weights (gw1/gw2) in fp8; False -> bf16, no DoubleRow

# packed f32 small-weight layout (columns of wf)
F_GB1 = 0
F_GB2 = 1
F_MB1 = 5
F_MB2 = 6        # 8 cols: gamma-bias [0:4], beta-bias [4:8]
F_ID = 14
F_IOTA = F_ID + 128
NF = F_IOTA + CG  # 146

# 2a split over 4096-wide chunks: ACT abs+accum, DVE ts+abs-reduce,
# GPSIMD abs-reduce of DVE-produced y tiles
ACT_2A = ((2, 0, L), (0, 0, L), (1, 0, L))   # (group, start-chunk, width)
DVE_2A = ((2, 1), (3, 0), (3, 1))

_CACHE = {}


def _build_nc():
    import concourse.bacc as bacc
    import concourse.bass as bass
    import concourse.tile as tile
    from concourse import mybir

    f32 = mybir.dt.float32
    bf16 = mybir.dt.bfloat16
    fp8 = mybir.dt.float8e4
    AF = mybir.ActivationFunctionType
    OP = mybir.AluOpType
    AX = mybir.AxisListType
    PM = mybir.MatmulPerfMode

    nc = bacc.Bacc("TRN2", target_bir_lowering=False, debug=False, num_devices=8)

    x_d = nc.declare_dram_parameter("x", [C, L], bf16, isOutput=False)
    c_d = nc.declare_dram_parameter("c", [C, L], fp8, isOutput=False)
    w1_d = nc.declare_dram_parameter("w1p", [128, CG * H], fp8 if W8 else bf16,
                                     isOutput=False)
    w2_d = nc.declare_dram_parameter("w2p", [128, CG * 128], fp8 if W8 else bf16,
                                     isOutput=False)
    m1_d = nc.declare_dram_parameter("m1p", [128, CG * H], bf16, isOutput=False)
    mw2_d = nc.declare_dram_parameter("mw2p", [128, 2 * C], f32, isOutput=False)
    wf_d = nc.declare_dram_parameter("wfp", [128, NF], f32, isOutput=False)
    out_d = nc.declare_dram_parameter("out", [C, L], bf16, isOutput=True)

    with tile.TileContext(nc) as tc:
        _emit(tc, bass, mybir, f32, bf16, fp8, AF, OP, AX, PM,
              x_d, c_d, w1_d, w2_d, m1_d, mw2_d, wf_d, out_d)

    nc.compile()
    return nc


def _emit(tc, bass, mybir, f32, bf16, fp8, AF, OP, AX, PM,
          x_d, c_d, w1_d, w2_d, m1_d, mw2_d, wf_d, out_d):
    from contextlib import ExitStack

    nc = tc.nc
    w8dt = fp8 if W8 else bf16

    with ExitStack() as top:
        xpool = top.enter_context(tc.tile_pool(name="xbuf", bufs=1))
        wpool = top.enter_context(tc.tile_pool(name="wts", bufs=1))
        spool = top.enter_context(tc.tile_pool(name="stats", bufs=1))
        ps_m = top.enter_context(tc.tile_pool(name="psm", bufs=1, space="PSUM"))
        m1_ps = ps_m.tile([128, LB], f32, tag="m1ps", name="m1ps")

        # ---- persistent tiles ----
        X = xpool.tile([128, CG, L], bf16, tag="X", name="X")
        w1 = wpool.tile([128, CG, H], w8dt, tag="w1", name="w1")
        w2 = wpool.tile([128, CG, 128], w8dt, tag="w2", name="w2")
        m1 = wpool.tile([128, CG, H], bf16, tag="m1", name="m1")
        mw2 = wpool.tile([128, 2 * C], f32, tag="mw2", name="mw2")
        wf = wpool.tile([128, NF], f32, tag="wf", name="wf")

        stats = [spool.tile([128, NLB, 6], f32, tag=f"bnst{g}", name=f"bnst{g}")
                 for g in range(CG)]
        gacc = spool.tile([128, CG, NST], f32, tag="gacc", name="gacc")
        impacc = spool.tile([128, CG, NYC], f32, tag="impacc", name="impacc")
        muvar = spool.tile([128, CG, 2], f32, tag="muvar", name="muvar")
        work = spool.tile([128, 16], f32, tag="work", name="work")
        scal = spool.tile([128, 8], f32, tag="scal", name="scal")
        sig5 = spool.tile([128, 5], f32, tag="sig5", name="sig5")
        gm4 = spool.tile([128, CG], f32, tag="gm4", name="gm4")
        mu4t = spool.tile([128, CG], f32, tag="mu4t", name="mu4t")
        sg4t = spool.tile([128, CG], f32, tag="sg4t", name="sg4t")
        imp4 = spool.tile([128, CG], f32, tag="imp4", name="imp4")
        A4 = spool.tile([128, CG], f32, tag="A4", name="A4")
        B4 = spool.tile([128, CG], f32, tag="B4", name="B4")
        rank4 = spool.tile([128, CG], f32, tag="rank4", name="rank4")
        mask4 = spool.tile([128, CG], f32, tag="mask4", name="mask4")
        Am4 = spool.tile([128, CG], f32, tag="Am4", name="Am4")
        Bm4 = spool.tile([128, CG], f32, tag="Bm4", name="Bm4")
        hm_sb = spool.tile([128, 1], f32, tag="hm", name="hm")
        sqscr = spool.tile([128, 1], f32, tag="sqscr", name="sqscr")
        tr_sb = spool.tile([1, CG, 128], f32, tag="tr4", name="tr4")
        G_sb = spool.tile([128, C], f32, tag="Gsb", name="Gsb")
        ones_sb = spool.tile([128, 128], f32, tag="ones", name="ones")
        zeros_sb = spool.tile([128, L], bf16, tag="zeros", name="zeros")
        idx_f = spool.tile([128, CG], f32, tag="idxf", name="idx_f")
        idx32 = spool.tile([128, CG], mybir.dt.int32, tag="idx32", name="idx32")
        rb_sb = spool.tile([128, CG, 4], bf16, tag="rb", name="rb_sb")

        nc.vector.memset(ones_sb[:], 1.0)
        nc.vector.memset(impacc[:], 0.0)
        nc.vector.memset(sqscr[:], 1.0)
        nc.gpsimd.memset(zeros_sb[:], 0.0)
        # pre-warm the sigmoid table set (includes relu) before any data lands
        nc.scalar.activation(out=sqscr[:], in_=sqscr[:], func=AF.Sigmoid,
                             bias=0.0, scale=1.0)

        gb1 = wf[:, F_GB1:F_GB1 + 1]
        mb1 = wf[:, F_MB1:F_MB1 + 1]
        ident_sb = wf[:, F_ID:F_ID + 128]

        # =========================== phase 1 ===========================
        with ExitStack() as ph1:
            cpool = ph1.enter_context(tc.tile_pool(name="cbuf", bufs=4))
            hpool = ph1.enter_context(tc.tile_pool(name="hbuf", bufs=2))
            gspool = ph1.enter_context(tc.tile_pool(name="gscr", bufs=4))
            ps_h = ph1.enter_context(tc.tile_pool(name="psh", bufs=1, space="PSUM"))
            ps_g = ph1.enter_context(tc.tile_pool(name="psg", bufs=2, space="PSUM"))

            cap = c_d[:]
            xap = x_d[:]

            def dma_c(c_t, k, half):
                l0 = k * XC + half * SL
                src = bass.AP(tensor=cap.tensor, offset=l0,
                              ap=[[L, 128], [128 * L, CG], [1, SL]])
                nc.sync.dma_start(out=c_t[:, :, half * SL:(half + 1) * SL],
                                  in_=src)

            def dma_x(k):
                l0 = k * XC
                src = bass.AP(tensor=xap.tensor, offset=l0,
                              ap=[[L, 128], [128 * L, CG], [1, XC]])
                nc.sync.dma_start(out=X[:, :, l0:l0 + XC], in_=src)

            for k in range(NXC):
                c_t = cpool.tile([128, CG, XC], fp8, tag="ct", name="ct")
                # DMA order: c runs one slot ahead of x (ACT/PE depend on c;
                # the DVE stats tolerate late x)
                if k == 0:
                    dma_c(c_t, 0, 0)
                    nc.sync.dma_start(out=w1[:], in_=w1_d[:])
                    nc.sync.dma_start(out=wf[:], in_=wf_d[:])
                    nc.sync.dma_start(out=m1[:], in_=m1_d[:])
                    nc.sync.dma_start(out=w2[:], in_=w2_d[:])
                    dma_c(c_t, 0, 1)
                else:
                    dma_c(c_t, k, 0)
                    dma_c(c_t, k, 1)
                    dma_x(k - 1)
                if k == 2:
                    nc.sync.dma_start(out=mw2[:], in_=mw2_d[:])
                if k == NXC - 1:
                    dma_x(NXC - 1)

                for half in range(2):
                    st = k * 2 + half
                    # gate layer 1: h = relu(gw1 @ c + gb1); relu on ACT
                    h_ps = ps_h.tile([128, SL], f32, tag="hps", name="hps")
                    for hh in range(2):
                        hs = slice(hh * LB, (hh + 1) * LB)
                        ds = slice(half * SL + hh * LB, half * SL + (hh + 1) * LB)
                        for g in range(CG):
                            nc.tensor.matmul(h_ps[:, hs], w1[:, g, :],
                                             c_t[:, g, ds],
                                             start=(g == 0), stop=(g == CG - 1))
                    h_sb = hpool.tile([128, SL], bf16, tag="hsb", name="hsb")
                    if k >= 1 and half == 1:
                        nc.vector.tensor_scalar(out=h_sb[:], in0=h_ps[:],
                                                scalar1=gb1, scalar2=0.0,
                                                op0=OP.add, op1=OP.max)
                    else:
                        nc.scalar.activation(out=h_sb[:], in_=h_ps[:],
                                             func=AF.Relu, bias=gb1, scale=1.0)

                    # cond-MLP layer 1: accumulate mw1 @ c into one PSUM bank
                    for g in range(CG):
                        for hh in range(2):
                            ds = slice(half * SL + hh * LB,
                                       half * SL + (hh + 1) * LB)
                            first = (k == 0 and half == 0 and g == 0 and hh == 0)
                            last = (k == NXC - 1 and half == 1
                                    and g == CG - 1 and hh == 1)
                            nc.tensor.matmul(m1_ps[:], m1[:, g, :],
                                             c_t[:, g, ds],
                                             start=first, stop=last)

                    # gate layer 2 + sigmoid (+ g_mix accumulator on ACT)
                    for g in range(CG):
                        g_ps = ps_g.tile([128, SL], f32, tag="gps", name="g_ps")
                        for hh in range(2):
                            hs = slice(hh * LB, (hh + 1) * LB)
                            nc.tensor.matmul(g_ps[:, hs], w2[:, g, :],
                                             h_sb[:, hs], start=True, stop=True)
                        g_scr = gspool.tile([128, SL], bf16, tag="gscr",
                                            name="gscr")
                        nc.scalar.activation(out=g_scr[:], in_=g_ps[:],
                                             func=AF.Sigmoid,
                                             bias=wf[:, F_GB2 + g:F_GB2 + g + 1],
                                             scale=1.0,
                                             accum_out=gacc[:, g, st:st + 1])

                # channel stats for this slot (DVE only runs bn_stats)
                if k > 0:
                    for g in range(CG):
                        for hh in range(NLB // NXC):
                            j = (k - 1) * (NLB // NXC) + hh
                            nc.vector.bn_stats(out=stats[g][:, j, :],
                                               in_=X[:, g, j * LB:(j + 1) * LB])
                if k == NXC - 1:
                    for g in range(CG):
                        for hh in range(NLB // NXC):
                            j = (NXC - 1) * (NLB // NXC) + hh
                            nc.vector.bn_stats(out=stats[g][:, j, :],
                                               in_=X[:, g, j * LB:(j + 1) * LB])

        # =========================== finalize ===========================
        with ExitStack() as fin:
            ps_f = fin.enter_context(tc.tile_pool(name="psf", bufs=1, space="PSUM"))

            # cond MLP head (deps ready before x stats): hm = relu(mean+mb1)
            nc.vector.reduce_sum(out=hm_sb[:], in_=m1_ps[:], axis=AX.X)
            # dummy sqrt pulls the sqrt table load off the critical path
            nc.scalar.activation(out=sqscr[:], in_=sqscr[:], func=AF.Sqrt,
                                 bias=0.0, scale=1.0)
            nc.scalar.activation(out=hm_sb[:], in_=hm_sb[:], func=AF.Relu,
                                 bias=mb1, scale=1.0 / L)
            gb_ps = ps_f.tile([128, 2 * CG], f32, tag="gbps", name="gbps")
            for o in range(2 * CG):
                nc.tensor.matmul(gb_ps[:, o:o + 1],
                                 mw2[:, o * 128:(o + 1) * 128],
                                 hm_sb[:], start=True, stop=True)

            for g in range(CG):
                nc.vector.bn_aggr(out=muvar[:, g, :], in_=stats[g][:])
            mu_c = muvar[:, :, 0]
            var_c = muvar[:, :, 1]
            nc.vector.tensor_copy(out=work[:, 0:4], in_=mu_c)
            # work 4:8 = E[x^2] = mu_c^2 + var_c
            nc.vector.tensor_tensor(out=work[:, 4:8], in0=mu_c, in1=mu_c,
                                    op=OP.mult)
            nc.vector.tensor_add(out=work[:, 4:8], in0=work[:, 4:8], in1=var_c)

            # fused cross-partition sum + broadcast: every partition gets
            # the 8 column sums
            cs_ps = ps_f.tile([128, 8], f32, tag="csps", name="csps")
            nc.tensor.matmul(cs_ps[:], ones_sb[:], work[:, 0:8],
                             start=True, stop=True)
            mu_l = scal[:, 0:1]
            ex2_l = scal[:, 1:2]
            var_l = scal[:, 2:3]
            nc.vector.reduce_sum(out=mu_l, in_=cs_ps[:, 0:4], axis=AX.X)
            nc.vector.tensor_scalar(out=mu_l, in0=mu_l, scalar1=1.0 / C,
                                    scalar2=None, op0=OP.mult)
            nc.vector.reduce_sum(out=ex2_l, in_=cs_ps[:, 4:8], axis=AX.X)
            nc.vector.tensor_scalar(out=ex2_l, in0=ex2_l, scalar1=1.0 / C,
                                    scalar2=None, op0=OP.mult)
            nc.vector.tensor_tensor(out=var_l, in0=mu_l, in1=mu_l, op=OP.mult)
            nc.vector.tensor_tensor(out=var_l, in0=ex2_l, in1=var_l,
                                    op=OP.subtract)

            # single 5-wide sqrt: [var_l, var_c0..3] (+eps)
            nc.vector.tensor_copy(out=work[:, 8:9], in_=var_l)
            nc.vector.tensor_copy(out=work[:, 9:13], in_=var_c)
            nc.vector.tensor_scalar(out=work[:, 8:13], in0=work[:, 8:13],
                                    scalar1=EPS, scalar2=None, op0=OP.add)
            nc.scalar.activation(out=sig5[:], in_=work[:, 8:13], func=AF.Sqrt,
                                 bias=0.0, scale=1.0)
            sig_l = sig5[:, 0:1]
            sig_c = sig5[:, 1:5]

            # g_mix
            nc.vector.tensor_reduce(out=gm4[:], in_=gacc[:], axis=AX.X, op=OP.add)
            nc.vector.tensor_scalar(out=gm4[:], in0=gm4[:], scalar1=1.0 / L,
                                    scalar2=None, op0=OP.mult)

            # mu = mu_l + g_mix*(mu_c - mu_l); sigma likewise
            nc.vector.tensor_scalar(out=mu4t[:], in0=mu_c, scalar1=mu_l,
                                    scalar2=None, op0=OP.subtract)
            nc.vector.tensor_tensor(out=mu4t[:], in0=mu4t[:], in1=gm4[:],
                                    op=OP.mult)
            nc.vector.tensor_scalar(out=mu4t[:], in0=mu4t[:], scalar1=mu_l,
                                    scalar2=None, op0=OP.add)
            nc.vector.tensor_scalar(out=sg4t[:], in0=sig_c, scalar1=sig_l,
                                    scalar2=None, op0=OP.subtract)
            nc.vector.tensor_tensor(out=sg4t[:], in0=sg4t[:], in1=gm4[:],
                                    op=OP.mult)
            nc.vector.tensor_scalar(out=sg4t[:], in0=sg4t[:], scalar1=sig_l,
                                    scalar2=None, op0=OP.add)

            # A = (1+gamma)/sigma ; B = beta - mu*A
            inv4 = work[:, 0:4]
            gam4 = work[:, 4:8]
            bet4 = work[:, 8:12]
            muA = work[:, 12:16]
            nc.vector.reciprocal(out=inv4, in_=sg4t[:])
            nc.vector.tensor_add(out=gam4, in0=gb_ps[:, 0:CG],
                                 in1=wf[:, F_MB2:F_MB2 + CG])
            nc.vector.tensor_scalar(out=gam4, in0=gam4, scalar1=1.0,
                                    scalar2=None, op0=OP.add)
            nc.vector.tensor_add(out=bet4, in0=gb_ps[:, CG:2 * CG],
                                 in1=wf[:, F_MB2 + CG:F_MB2 + 2 * CG])
            nc.vector.tensor_tensor(out=A4[:], in0=gam4, in1=inv4, op=OP.mult)
            nc.vector.tensor_tensor(out=muA, in0=mu4t[:], in1=A4[:], op=OP.mult)
            nc.vector.tensor_tensor(out=B4[:], in0=bet4, in1=muA, op=OP.subtract)

        # =========================== phase 2 ===========================
        # Fused output: stream UNMASKED y = A*x+B to HBM immediately (out-DMA
        # overlaps all remaining compute), accumulate imp = sum|y| on ACT/DVE
        # meanwhile, rank channels, then scatter ZERO rows over the dropped
        # channels via one indirect DMA per group (OOB indices skip kept rows).
        with ExitStack() as ph2:
            ypool = ph2.enter_context(tc.tile_pool(name="ybuf", bufs=3))
            apool = ph2.enter_context(tc.tile_pool(name="abuf", bufs=1))
            ps_t = ph2.enter_context(tc.tile_pool(name="pst", bufs=1, space="PSUM"))

            tr_ps = ps_t.tile([1, CG, 128], f32, tag="trps", name="trps")
            T_ps = ps_t.tile([128, C], f32, tag="Tps", name="Tps")

            def imp_bcast(g):
                # fold this group's accumulators and broadcast its 128 imps
                # into T_ps columns [g*128, (g+1)*128)
                nc.vector.tensor_reduce(out=imp4[:, g:g + 1],
                                        in_=impacc[:, g, :], axis=AX.X,
                                        op=OP.add)
                nc.tensor.matmul(tr_ps[0:1, g, :], imp4[:, g:g + 1],
                                 ident_sb, is_transpose=True,
                                 start=True, stop=True)
                nc.vector.tensor_copy(out=tr_sb[0:1, g, :], in_=tr_ps[0:1, g, :])
                nc.tensor.matmul(T_ps[:, g * 128:(g + 1) * 128],
                                 ones_sb[0:1, 0:128], tr_sb[0:1, g, :],
                                 start=True, stop=True)

            # unmasked y stream, g3 first: DVE ts -> 2-MiB DMA -> tiny
            # read-back fence per group (FIFO on the sync queue guarantees the
            # fence completes only after that group's y rows are in HBM)
            y_ts = {}

            def stream_group(g):
                y_t = ypool.tile([128, L], bf16, tag="yt", name="yt")
                for h in range(2):
                    hs = slice(h * YC, (h + 1) * YC)
                    nc.vector.tensor_scalar(out=y_t[:, hs], in0=X[:, g, hs],
                                            scalar1=A4[:, g:g + 1],
                                            scalar2=B4[:, g:g + 1],
                                            op0=OP.mult, op1=OP.add)
                    nc.sync.dma_start(
                        out=out_d[g * 128:(g + 1) * 128, hs],
                        in_=y_t[:, hs])
                nc.sync.dma_start(out=rb_sb[:, g, :],
                                  in_=out_d[g * 128:(g + 1) * 128, 0:4])
                y_ts[g] = y_t

            def act_abs(g, j, w):
                scr = apool.tile([128, L], bf16, tag="ascr", name="ascr")
                nc.scalar.activation(out=scr[:, 0:w],
                                     in_=X[:, g, j * YC:j * YC + w],
                                     func=AF.Abs,
                                     bias=B4[:, g:g + 1],
                                     scale=A4[:, g:g + 1],
                                     accum_out=impacc[:, g, j:j + 1])

            stream_group(3)
            stream_group(2)
            for args in ACT_2A:
                act_abs(*args)
            # DVE abs of g3's streamed tile (before ts(g1): pool-safe)
            nc.vector.tensor_reduce(out=impacc[:, 3, 0:1], in_=y_ts[3][:],
                                    axis=AX.X, op=OP.add,
                                    apply_absolute_value=True)
            imp_bcast(3)
            stream_group(1)
            stream_group(0)
            imp_bcast(2)
            imp_bcast(0)
            imp_bcast(1)

            # rank by count of larger imps; mask = rank < KEEP
            for g in range(CG):
                nc.vector.tensor_scalar(out=G_sb[:], in0=T_ps[:],
                                        scalar1=imp4[:, g:g + 1], scalar2=0.0,
                                        op0=OP.is_gt, op1=OP.add,
                                        accum_out=rank4[:, g:g + 1])
            nc.vector.tensor_scalar(out=mask4[:], in0=rank4[:],
                                    scalar1=float(KEEP), scalar2=None,
                                    op0=OP.is_lt)

            # scatter indices: dropped channels keep their row id, kept
            # channels get an out-of-bounds id (skipped by bounds_check);
            # each group's column is fenced on that group's own read-back so
            # early groups' scatters need not wait for the whole stream
            nc.vector.tensor_scalar(out=idx_f[:], in0=mask4[:],
                                    scalar1=float(4 * C), scalar2=None,
                                    op0=OP.mult)
            nc.vector.tensor_add(out=idx_f[:], in0=idx_f[:],
                                 in1=wf[:, F_IOTA:F_IOTA + CG])
            for g in (3, 2, 1, 0):
                nc.vector.tensor_scalar(out=scal[:, 6:7], in0=rb_sb[:, g, 0:1],
                                        scalar1=0.0, scalar2=None, op0=OP.mult)
                nc.vector.tensor_scalar(out=idx_f[:, g:g + 1],
                                        in0=idx_f[:, g:g + 1],
                                        scalar1=scal[:, 6:7], scalar2=None,
                                        op0=OP.add)
                nc.vector.tensor_copy(out=idx32[:, g:g + 1],
                                      in_=idx_f[:, g:g + 1])
                nc.gpsimd.indirect_dma_start(
                    out=out_d[:],
                    out_offset=bass.IndirectOffsetOnAxis(ap=idx32[:, g:g + 1],
                                                         axis=0),
                    in_=zeros_sb[:],
                    in_offset=None,
                    bounds_check=C - 1,
                    oob_is_err=False)


def _get_nc():
    if "nc" not in _CACHE:
        _CACHE["nc"] = _build_nc()
    return _CACHE["nc"]


def _ef_cast_fp8(c, f8):
    """Cast [B,C,L] fp32 -> fp8 with per-(b,c) error-feedback along L, so the
    running sum of the quantized stream tracks the fp32 sum to ~1 ulp (keeps
    cond_pool = mean_L(c) accurate despite fp8 storage)."""
    c = np.asarray(c, np.float32)
    Bc, Cc, Lc = c.shape
    flat = c.reshape(Bc * Cc, Lc)
    out = np.empty((Bc * Cc, Lc), dtype=f8)
    carry = np.zeros((Bc * Cc,), np.float32)
    for l in range(Lc):
        v = flat[:, l] + carry
        q = v.astype(f8)
        out[:, l] = q
        carry = v - q.astype(np.float32)
    return out.reshape(Bc, Cc, Lc)


def _host_weight_maps(gw1, gb1, gw2, gb2, mw1, mb1, mw2, mb2):
    import ml_dtypes
    f = np.float32
    bf = ml_dtypes.bfloat16
    f8 = ml_dtypes.float8_e4m3
    w8 = f8 if W8 else bf
    # [C,H] -> [128, CG, H] (c-within-group on partitions, group-major free)
    w1t = np.asarray(gw1, f).T.reshape(CG, 128, H).transpose(1, 0, 2)
    m1t = np.asarray(mw1, f).T.reshape(CG, 128, H).transpose(1, 0, 2)
    w2t = np.asarray(gw2, f).T                                  # [H, C]
    wpk_f = np.zeros((128, NF), f)
    wpk_f[:, F_GB1] = np.asarray(gb1, f)
    wpk_f[:, F_GB2:F_GB2 + CG] = np.asarray(gb2, f).reshape(CG, 128).T
    wpk_f[:, F_MB1] = np.asarray(mb1, f)
    wpk_f[:, F_MB2:F_MB2 + 2 * CG] = np.asarray(mb2, f).reshape(2 * CG, 128).T
    wpk_f[:, F_ID:F_ID + 128] = np.eye(128, dtype=f)
    iota = np.arange(128, dtype=f)
    for g in range(CG):
        wpk_f[:, F_IOTA + g] = g * 128 + iota
    return {
        "w1p": np.ascontiguousarray(w1t.reshape(128, CG * H).astype(w8)),
        "w2p": np.ascontiguousarray(w2t.astype(w8)),
        "m1p": np.ascontiguousarray(m1t.reshape(128, CG * H).astype(bf)),
        "mw2p": np.ascontiguousarray(np.asarray(mw2, f).T),
        "wfp": np.ascontiguousarray(wpk_f),
    }


def _run(inputs, trace=False):
    import ml_dtypes
    from concourse.bass_utils import run_bass_kernel_spmd

    nc = _get_nc()
    bf = ml_dtypes.bfloat16
    f8 = ml_dtypes.float8_e4m3
    x = np.asarray(inputs["x"], np.float32).astype(bf)
    c8 = _ef_cast_fp8(inputs["c"], f8)
    wmap = _host_weight_maps(
        inputs["gw1"], inputs["gb1"], inputs["gw2"], inputs["gb2"],
        inputs["mw1"], inputs["mb1"], inputs["mw2"], inputs["mb2"])
    in_maps = [
        dict(wmap, x=np.ascontiguousarray(x[b]), c=np.ascontiguousarray(c8[b]))
        for b in range(B)
    ]
    res = run_bass_kernel_spmd(nc, in_maps, core_ids=list(range(B)), trace=trace)
    out = np.stack([np.asarray(res.results[b]["out"], np.float32) for b in range(B)],
                   axis=0)
    return out, res


def kernel(**inputs):
    out, _ = _run(inputs, trace=False)
    return out


# revision 34
# speedup vs baseline: 1.0183x; 1.0183x over previous
"""Trainium2 Bass kernel for the DUAN conditioned-normalization problem.

Contract: kernel(**inputs) takes FULL inputs (B=8 samples), shards one sample
per NeuronCore (8 cores), runs a single Bass/Tile kernel SPMD, and gathers the
full [8, 512, 8192] output.

Per-sample math (matches the jax reference):
  mu_c/var_c over L per channel; mu_l/var_l over (C,L);
  g = sigmoid(gw2 @ relu(gw1 @ c + gb1) + gb2); g_mix = mean_L(g)
  gamma,beta = mw2 @ relu(mw1 @ mean_L(c) + mb1) + mb2
  mu = g_mix*mu_c + (1-g_mix)*mu_l ; sigma likewise from sqrt(var+eps)
  y = (1+gamma)*(x-mu)/sigma + beta ; keep top-k channels by mean_L |y|.

V11 layout (x bf16, c fp8-e4m3 with per-channel error-feedback rounding so
mean_L(c8) stays exact; out bf16 -> ~21 MiB HBM/core):
  Phase 1 streams c one slot ahead of x; PE runs both gate layers (fp8
  weights) and accumulates bf16-mw1 @ fp8-c into one PSUM bank (cond-MLP
  pooling for free); ACT runs the sigmoid with g-mean accumulators plus six
  of eight gate relus (table set pre-warmed at t=0); DVE runs bn_stats
  channel stats plus the other two relus.  Finalize fuses cross-partition
  sum+broadcast in one ones-matmul, uses a single 5-wide sqrt (table
  preloaded by a dummy op), and folds everything into per-channel
  A=(1+gamma)/sigma, B=beta-mu*A.  Phase 2 streams UNMASKED y=A*x+B to HBM
  as 2-MiB tiles immediately (out-DMA hides the imp work); imp=sum|y|
  accumulates meanwhile on ACT (Abs+accum from x) and DVE (abs-reduce of the
  streamed tiles); channels are ranked by count against a PE-broadcast fp32
  imp matrix; finally one indirect DMA per group scatters zero rows over the
  dropped channels (kept channels get out-of-bounds indices, skipped via
  bounds_check), fenced behind the y writes by a same-queue read-back.
"""

import sys

sys.path.insert(0, "/opt/trn_rl_repo")

import numpy as np

B = 8
C = 512
L = 8192
H = 128
CG = 4           # channel groups of 128 partitions
SL = 1024        # phase-1 supertile width along L (gate granularity)
NST = L // SL    # 8
LB = 512         # bn_stats / matmul block
NLB = L // LB    # 16
XC = 2048        # phase-1 slot width along L
NXC = L // XC    # 4
YC = 4096        # phase-2 chunk
NYC = L // YC    # 2
KEEP = max(1, int(C * 0.7))  # 358
EPS = 1e-5

W8 = True        # gate weights (gw1/gw2) in fp8; False -> bf16, no DoubleRow

# packed f32 small-weight layout (columns of wf)
F_GB1 = 0
F_GB2 = 1
F_MB1 = 5
F_MB2 = 6        # 8 cols: gamma-bias [0:4], beta-bias [4:8]
F_ID = 14
F_IOTA = F_ID + 128
NF = F_IOTA + CG  # 146

# 2a split over 4096-wide chunks: ACT abs+accum, DVE ts+abs-reduce,
# GPSIMD abs-reduce of DVE-produced y tiles
ACT_2A = ((2, 0, L), (0, 0, L), (1, 0, 3 * YC // 2))   # (group, start, width)
DVE_2A = ((2, 1), (3, 0), (3, 1))

_CACHE = {}


def _build_nc():
    import concourse.bacc as bacc
    import concourse.bass as bass
    import concourse.tile as tile
    from concourse import mybir

    f32 = mybir.dt.float32
    bf16 = mybir.dt.bfloat16
    fp8 = mybir.dt.float8e4
    AF = mybir.ActivationFunctionType
    OP = mybir.AluOpType
    AX = mybir.AxisListType
    PM = mybir.MatmulPerfMode

    nc = bacc.Bacc("TRN2", target_bir_lowering=False, debug=False, num_devices=8)

    x_d = nc.declare_dram_parameter("x", [C, L], bf16, isOutput=False)
    c_d = nc.declare_dram_parameter("c", [C, L], fp8, isOutput=False)
    w1_d = nc.declare_dram_parameter("w1p", [128, CG * H], fp8 if W8 else bf16,
                                     isOutput=False)
    w2_d = nc.declare_dram_parameter("w2p", [128, CG * 128], fp8 if W8 else bf16,
                                     isOutput=False)
    m1_d = nc.declare_dram_parameter("m1p", [128, CG * H], bf16, isOutput=False)
    mw2_d = nc.declare_dram_parameter("mw2p", [128, 2 * C], f32, isOutput=False)
    wf_d = nc.declare_dram_parameter("wfp", [128, NF], f32, isOutput=False)
    out_d = nc.declare_dram_parameter("out", [C, L], bf16, isOutput=True)

    with tile.TileContext(nc) as tc:
        _emit(tc, bass, mybir, f32, bf16, fp8, AF, OP, AX, PM,
              x_d, c_d, w1_d, w2_d, m1_d, mw2_d, wf_d, out_d)

    nc.compile()
    return nc


def _emit(tc, bass, mybir, f32, bf16, fp8, AF, OP, AX, PM,
          x_d, c_d, w1_d, w2_d, m1_d, mw2_d, wf_d, out_d):
    from contextlib import ExitStack

    nc = tc.nc
    w8dt = fp8 if W8 else bf16

    with ExitStack() as top:
        xpool = top.enter_context(tc.tile_pool(name="xbuf", bufs=1))
        wpool = top.enter_context(tc.tile_pool(name="wts", bufs=1))
        spool = top.enter_context(tc.tile_pool(name="stats", bufs=1))
        ps_m = top.enter_context(tc.tile_pool(name="psm", bufs=1, space="PSUM"))
        m1_ps = ps_m.tile([128, LB], f32, tag="m1ps", name="m1ps")

        # ---- persistent tiles ----
        X = xpool.tile([128, CG, L], bf16, tag="X", name="X")
        w1 = wpool.tile([128, CG, H], w8dt, tag="w1", name="w1")
        w2 = wpool.tile([128, CG, 128], w8dt, tag="w2", name="w2")
        m1 = wpool.tile([128, CG, H], bf16, tag="m1", name="m1")
        mw2 = wpool.tile([128, 2 * C], f32, tag="mw2", name="mw2")
        wf = wpool.tile([128, NF], f32, tag="wf", name="wf")

        stats = [spool.tile([128, NLB, 6], f32, tag=f"bnst{g}", name=f"bnst{g}")
                 for g in range(CG)]
        gacc = spool.tile([128, CG, NST], f32, tag="gacc", name="gacc")
        impacc = spool.tile([128, CG, NYC], f32, tag="impacc", name="impacc")
        muvar = spool.tile([128, CG, 2], f32, tag="muvar", name="muvar")
        work = spool.tile([128, 16], f32, tag="work", name="work")
        scal = spool.tile([128, 8], f32, tag="scal", name="scal")
        sig5 = spool.tile([128, 5], f32, tag="sig5", name="sig5")
        gm4 = spool.tile([128, CG], f32, tag="gm4", name="gm4")
        mu4t = spool.tile([128, CG], f32, tag="mu4t", name="mu4t")
        sg4t = spool.tile([128, CG], f32, tag="sg4t", name="sg4t")
        imp4 = spool.tile([128, CG], f32, tag="imp4", name="imp4")
        A4 = spool.tile([128, CG], f32, tag="A4", name="A4")
        B4 = spool.tile([128, CG], f32, tag="B4", name="B4")
        rank4 = spool.tile([128, CG], f32, tag="rank4", name="rank4")
        mask4 = spool.tile([128, CG], f32, tag="mask4", name="mask4")
        Am4 = spool.tile([128, CG], f32, tag="Am4", name="Am4")
        Bm4 = spool.tile([128, CG], f32, tag="Bm4", name="Bm4")
        hm_sb = spool.tile([128, 1], f32, tag="hm", name="hm")
        sqscr = spool.tile([128, 1], f32, tag="sqscr", name="sqscr")
        tr_sb = spool.tile([1, CG, 128], f32, tag="tr4", name="tr4")
        G_sb = spool.tile([128, C], f32, tag="Gsb", name="Gsb")
        ones_sb = spool.tile([128, 128], f32, tag="ones", name="ones")
        zeros_sb = spool.tile([128, L], bf16, tag="zeros", name="zeros")
        idx_f = spool.tile([128, CG], f32, tag="idxf", name="idx_f")
        idx32 = spool.tile([128, CG], mybir.dt.int32, tag="idx32", name="idx32")
        rb_sb = spool.tile([128, CG, 4], bf16, tag="rb", name="rb_sb")

        nc.vector.memset(ones_sb[:], 1.0)
        nc.vector.memset(impacc[:], 0.0)
        nc.vector.memset(sqscr[:], 1.0)
        nc.gpsimd.memset(zeros_sb[:], 0.0)
        # pre-warm the sigmoid table set (includes relu) before any data lands
        nc.scalar.activation(out=sqscr[:], in_=sqscr[:], func=AF.Sigmoid,
                             bias=0.0, scale=1.0)

        gb1 = wf[:, F_GB1:F_GB1 + 1]
        mb1 = wf[:, F_MB1:F_MB1 + 1]
        ident_sb = wf[:, F_ID:F_ID + 128]

        # =========================== phase 1 ===========================
        with ExitStack() as ph1:
            cpool = ph1.enter_context(tc.tile_pool(name="cbuf", bufs=4))
            hpool = ph1.enter_context(tc.tile_pool(name="hbuf", bufs=2))
            gspool = ph1.enter_context(tc.tile_pool(name="gscr", bufs=4))
            ps_h = ph1.enter_context(tc.tile_pool(name="psh", bufs=1, space="PSUM"))
            ps_g = ph1.enter_context(tc.tile_pool(name="psg", bufs=2, space="PSUM"))

            cap = c_d[:]
            xap = x_d[:]

            def dma_c(c_t, k, half):
                l0 = k * XC + half * SL
                src = bass.AP(tensor=cap.tensor, offset=l0,
                              ap=[[L, 128], [128 * L, CG], [1, SL]])
                nc.sync.dma_start(out=c_t[:, :, half * SL:(half + 1) * SL],
                                  in_=src)

            def dma_x(k):
                l0 = k * XC
                src = bass.AP(tensor=xap.tensor, offset=l0,
                              ap=[[L, 128], [128 * L, CG], [1, XC]])
                nc.sync.dma_start(out=X[:, :, l0:l0 + XC], in_=src)

            for k in range(NXC):
                c_t = cpool.tile([128, CG, XC], fp8, tag="ct", name="ct")
                # DMA order: c runs one slot ahead of x (ACT/PE depend on c;
                # the DVE stats tolerate late x)
                if k == 0:
                    dma_c(c_t, 0, 0)
                    nc.sync.dma_start(out=w1[:], in_=w1_d[:])
                    nc.sync.dma_start(out=wf[:], in_=wf_d[:])
                    nc.sync.dma_start(out=m1[:], in_=m1_d[:])
                    nc.sync.dma_start(out=w2[:], in_=w2_d[:])
                    dma_c(c_t, 0, 1)
                else:
                    dma_c(c_t, k, 0)
                    dma_c(c_t, k, 1)
                    dma_x(k - 1)
                if k == 2:
                    nc.sync.dma_start(out=mw2[:], in_=mw2_d[:])
                if k == NXC - 1:
                    dma_x(NXC - 1)

                for half in range(2):
                    st = k * 2 + half
                    # gate layer 1: h = relu(gw1 @ c + gb1); relu on ACT
                    h_ps = ps_h.tile([128, SL], f32, tag="hps", name="hps")
                    for hh in range(2):
                        hs = slice(hh * LB, (hh + 1) * LB)
                        ds = slice(half * SL + hh * LB, half * SL + (hh + 1) * LB)
                        for g in range(CG):
                            nc.tensor.matmul(h_ps[:, hs], w1[:, g, :],
                                             c_t[:, g, ds],
                                             start=(g == 0), stop=(g == CG - 1))
                    h_sb = hpool.tile([128, SL], bf16, tag="hsb", name="hsb")
                    if k >= 1 and half == 1:
                        nc.vector.tensor_scalar(out=h_sb[:], in0=h_ps[:],
                                                scalar1=gb1, scalar2=0.0,
                                                op0=OP.add, op1=OP.max)
                    else:
                        nc.scalar.activation(out=h_sb[:], in_=h_ps[:],
                                             func=AF.Relu, bias=gb1, scale=1.0)

                    # cond-MLP layer 1: accumulate mw1 @ c into one PSUM bank
                    for g in range(CG):
                        for hh in range(2):
                            ds = slice(half * SL + hh * LB,
                                       half * SL + (hh + 1) * LB)
                            first = (k == 0 and half == 0 and g == 0 and hh == 0)
                            last = (k == NXC - 1 and half == 1
                                    and g == CG - 1 and hh == 1)
                            nc.tensor.matmul(m1_ps[:], m1[:, g, :],
                                             c_t[:, g, ds],
                                             start=first, stop=last)

                    # gate layer 2 + sigmoid (+ g_mix accumulator on ACT)
                    for g in range(CG):
                        g_ps = ps_g.tile([128, SL], f32, tag="gps", name="g_ps")
                        for hh in range(2):
                            hs = slice(hh * LB, (hh + 1) * LB)
                            nc.tensor.matmul(g_ps[:, hs], w2[:, g, :],
                                             h_sb[:, hs], start=True, stop=True)
                        g_scr = gspool.tile([128, SL], bf16, tag="gscr",
                                            name="gscr")
                        nc.scalar.activation(out=g_scr[:], in_=g_ps[:],
                                             func=AF.Sigmoid,
                                             bias=wf[:, F_GB2 + g:F_GB2 + g + 1],
                                             scale=1.0,
                                             accum_out=gacc[:, g, st:st + 1])

                # channel stats for this slot (DVE only runs bn_stats)
                if k > 0:
                    for g in range(CG):
                        for hh in range(NLB // NXC):
                            j = (k - 1) * (NLB // NXC) + hh
                            nc.vector.bn_stats(out=stats[g][:, j, :],
                                               in_=X[:, g, j * LB:(j + 1) * LB])
                if k == NXC - 1:
                    for g in range(CG):
                        for hh in range(NLB // NXC):
                            j = (NXC - 1) * (NLB // NXC) + hh
                            nc.vector.bn_stats(out=stats[g][:, j, :],
                                               in_=X[:, g, j * LB:(j + 1) * LB])

        # =========================== finalize ===========================
        with ExitStack() as fin:
            ps_f = fin.enter_context(tc.tile_pool(name="psf", bufs=1, space="PSUM"))

            # cond MLP head (deps ready before x stats): hm = relu(mean+mb1)
            nc.vector.reduce_sum(out=hm_sb[:], in_=m1_ps[:], axis=AX.X)
            # dummy sqrt pulls the sqrt table load off the critical path
            nc.scalar.activation(out=sqscr[:], in_=sqscr[:], func=AF.Sqrt,
                                 bias=0.0, scale=1.0)
            nc.scalar.activation(out=hm_sb[:], in_=hm_sb[:], func=AF.Relu,
                                 bias=mb1, scale=1.0 / L)
            gb_ps = ps_f.tile([128, 2 * CG], f32, tag="gbps", name="gbps")
            for o in range(2 * CG):
                nc.tensor.matmul(gb_ps[:, o:o + 1],
                                 mw2[:, o * 128:(o + 1) * 128],
                                 hm_sb[:], start=True, stop=True)

            for g in range(CG):
                nc.vector.bn_aggr(out=muvar[:, g, :], in_=stats[g][:])
            mu_c = muvar[:, :, 0]
            var_c = muvar[:, :, 1]
            nc.vector.tensor_copy(out=work[:, 0:4], in_=mu_c)
            # work 4:8 = E[x^2] = mu_c^2 + var_c
            nc.vector.tensor_tensor(out=work[:, 4:8], in0=mu_c, in1=mu_c,
                                    op=OP.mult)
            nc.vector.tensor_add(out=work[:, 4:8], in0=work[:, 4:8], in1=var_c)

            # fused cross-partition sum + broadcast: every partition gets
            # the 8 column sums
            cs_ps = ps_f.tile([128, 8], f32, tag="csps", name="csps")
            nc.tensor.matmul(cs_ps[:], ones_sb[:], work[:, 0:8],
                             start=True, stop=True)
            mu_l = scal[:, 0:1]
            ex2_l = scal[:, 1:2]
            var_l = scal[:, 2:3]
            nc.vector.reduce_sum(out=mu_l, in_=cs_ps[:, 0:4], axis=AX.X)
            nc.vector.tensor_scalar(out=mu_l, in0=mu_l, scalar1=1.0 / C,
                                    scalar2=None, op0=OP.mult)
            nc.vector.reduce_sum(out=ex2_l, in_=cs_ps[:, 4:8], axis=AX.X)
            nc.vector.tensor_scalar(out=ex2_l, in0=ex2_l, scalar1=1.0 / C,
                                    scalar2=None, op0=OP.mult)
            nc.vector.tensor_tensor(out=var_l, in0=mu_l, in1=mu_l, op=OP.mult)
            nc.vector.tensor_tensor(out=var_l, in0=ex2_l, in1=var_l,
                                    op=OP.subtract)

            # single 5-wide sqrt: [var_l, var_c0..3] (+eps)
            nc.vector.tensor_copy(out=work[:, 8:9], in_=var_l)
            nc.vector.tensor_copy(out=work[:, 9:13], in_=var_c)
            nc.vector.tensor_scalar(out=work[:, 8:13], in0=work[:, 8:13],
                                    scalar1=EPS, scalar2=None, op0=OP.add)
            nc.scalar.activation(out=sig5[:], in_=work[:, 8:13], func=AF.Sqrt,
                                 bias=0.0, scale=1.0)
            sig_l = sig5[:, 0:1]
            sig_c = sig5[:, 1:5]

            # g_mix
            nc.vector.tensor_reduce(out=gm4[:], in_=gacc[:], axis=AX.X, op=OP.add)
            nc.vector.tensor_scalar(out=gm4[:], in0=gm4[:], scalar1=1.0 / L,
                                    scalar2=None, op0=OP.mult)

            # mu = mu_l + g_mix*(mu_c - mu_l); sigma likewise
            nc.vector.tensor_scalar(out=mu4t[:], in0=mu_c, scalar1=mu_l,
                                    scalar2=None, op0=OP.subtract)
            nc.vector.tensor_tensor(out=mu4t[:], in0=mu4t[:], in1=gm4[:],
                                    op=OP.mult)
            nc.vector.tensor_scalar(out=mu4t[:], in0=mu4t[:], scalar1=mu_l,
                                    scalar2=None, op0=OP.add)
            nc.vector.tensor_scalar(out=sg4t[:], in0=sig_c, scalar1=sig_l,
                                    scalar2=None, op0=OP.subtract)
            nc.vector.tensor_tensor(out=sg4t[:], in0=sg4t[:], in1=gm4[:],
                                    op=OP.mult)
            nc.vector.tensor_scalar(out=sg4t[:], in0=sg4t[:], scalar1=sig_l,
                                    scalar2=None, op0=OP.add)

            # A = (1+gamma)/sigma ; B = beta - mu*A
            inv4 = work[:, 0:4]
            gam4 = work[:, 4:8]
            bet4 = work[:, 8:12]
            muA = work[:, 12:16]
            nc.vector.reciprocal(out=inv4, in_=sg4t[:])
            nc.vector.tensor_add(out=gam4, in0=gb_ps[:, 0:CG],
                                 in1=wf[:, F_MB2:F_MB2 + CG])
            nc.vector.tensor_scalar(out=gam4, in0=gam4, scalar1=1.0,
                                    scalar2=None, op0=OP.add)
            nc.vector.tensor_add(out=bet4, in0=gb_ps[:, CG:2 * CG],
                                 in1=wf[:, F_MB2 + CG:F_MB2 + 2 * CG])
            nc.vector.tensor_tensor(out=A4[:], in0=gam4, in1=inv4, op=OP.mult)
            nc.vector.tensor_tensor(out=muA, in0=mu4t[:], in1=A4[:], op=OP.mult)
            nc.vector.tensor_tensor(out=B4[:], in0=bet4, in1=muA, op=OP.subtract)

        # =========================== phase 2 ===========================
        # Fused output: stream UNMASKED y = A*x+B to HBM immediately (out-DMA
        # overlaps all remaining compute), accumulate imp = sum|y| on ACT/DVE
        # meanwhile, rank channels, then scatter ZERO rows over the dropped
        # channels via one indirect DMA per group (OOB indices skip kept rows).
        with ExitStack() as ph2:
            ypool = ph2.enter_context(tc.tile_pool(name="ybuf", bufs=3))
            apool = ph2.enter_context(tc.tile_pool(name="abuf", bufs=1))
            ps_t = ph2.enter_context(tc.tile_pool(name="pst", bufs=1, space="PSUM"))

            tr_ps = ps_t.tile([1, CG, 128], f32, tag="trps", name="trps")
            T_ps = ps_t.tile([128, C], f32, tag="Tps", name="Tps")

            def imp_bcast(g):
                # fold this group's accumulators and broadcast its 128 imps
                # into T_ps columns [g*128, (g+1)*128)
                nc.vector.tensor_reduce(out=imp4[:, g:g + 1],
                                        in_=impacc[:, g, :], axis=AX.X,
                                        op=OP.add)
                nc.tensor.matmul(tr_ps[0:1, g, :], imp4[:, g:g + 1],
                                 ident_sb, is_transpose=True,
                                 start=True, stop=True)
                nc.vector.tensor_copy(out=tr_sb[0:1, g, :], in_=tr_ps[0:1, g, :])
                nc.tensor.matmul(T_ps[:, g * 128:(g + 1) * 128],
                                 ones_sb[0:1, 0:128], tr_sb[0:1, g, :],
                                 start=True, stop=True)

            # unmasked y stream, g3 first: DVE ts -> 2-MiB DMA -> tiny
            # read-back fence per group (FIFO on the sync queue guarantees the
            # fence completes only after that group's y rows are in HBM)
            y_ts = {}

            def stream_group(g):
                y_t = ypool.tile([128, L], bf16, tag="yt", name="yt")
                for h in range(2):
                    hs = slice(h * YC, (h + 1) * YC)
                    nc.vector.tensor_scalar(out=y_t[:, hs], in0=X[:, g, hs],
                                            scalar1=A4[:, g:g + 1],
                                            scalar2=B4[:, g:g + 1],
                                            op0=OP.mult, op1=OP.add)
                    nc.sync.dma_start(
                        out=out_d[g * 128:(g + 1) * 128, hs],
                        in_=y_t[:, hs])
                nc.sync.dma_start(out=rb_sb[:, g, :],
                                  in_=out_d[g * 128:(g + 1) * 128, 0:4])
                y_ts[g] = y_t

            def act_abs(g, j, w):
                scr = apool.tile([128, L], bf16, tag="ascr", name="ascr")
                nc.scalar.activation(out=scr[:, 0:w],
                                     in_=X[:, g, j * YC:j * YC + w],
                                     func=AF.Abs,
                                     bias=B4[:, g:g + 1],
                                     scale=A4[:, g:g + 1],
                                     accum_out=impacc[:, g, j:j + 1])

            stream_group(3)
            stream_group(2)
            for args in ACT_2A:
                act_abs(*args)
            # DVE abs of g3's streamed tile (before ts(g1): pool-safe)
            nc.vector.tensor_reduce(out=impacc[:, 3, 0:1], in_=y_ts[3][:],
                                    axis=AX.X, op=OP.add,
                                    apply_absolute_value=True)
            imp_bcast(3)
            stream_group(1)
            # DVE abs of g1's last quarter (balances the ACT abs path)
            nc.vector.tensor_reduce(out=impacc[:, 1, 1:2],
                                    in_=y_ts[1][:, 3 * YC // 2:], axis=AX.X,
                                    op=OP.add, apply_absolute_value=True)
            stream_group(0)
            imp_bcast(2)
            imp_bcast(0)
            imp_bcast(1)

            # rank by count of larger imps; mask = rank < KEEP
            for g in range(CG):
                nc.vector.tensor_scalar(out=G_sb[:], in0=T_ps[:],
                                        scalar1=imp4[:, g:g + 1], scalar2=0.0,
                                        op0=OP.is_gt, op1=OP.add,
                                        accum_out=rank4[:, g:g + 1])
            nc.vector.tensor_scalar(out=mask4[:], in0=rank4[:],
                                    scalar1=float(KEEP), scalar2=None,
                                    op0=OP.is_lt)

            # scatter indices: dropped channels keep their row id, kept
            # channels get an out-of-bounds id (skipped by bounds_check);
            # each group's column is fenced on that group's own read-back so
            # early groups' scatters need not wait for the whole stream
            nc.vector.tensor_scalar(out=idx_f[:], in0=mask4[:],
                                    scalar1=float(4 * C), scalar2=None,
                                    op0=OP.mult)
            nc.vector.tensor_add(out=idx_f[:], in0=idx_f[:],
                                 in1=wf[:, F_IOTA:F_IOTA + CG])
            for g in (3, 2, 1, 0):
                nc.vector.tensor_scalar(out=scal[:, 6:7], in0=rb_sb[:, g, 0:1],
                                        scalar1=0.0, scalar2=None, op0=OP.mult)
                nc.vector.tensor_scalar(out=idx_f[:, g:g + 1],
                                        in0=idx_f[:, g:g + 1],
                                        scalar1=scal[:, 6:7], scalar2=None,
                                        op0=OP.add)
                nc.vector.tensor_copy(out=idx32[:, g:g + 1],
                                      in_=idx_f[:, g:g + 1])
                nc.gpsimd.indirect_dma_start(
                    out=out_d[:],
                    out_offset=bass.IndirectOffsetOnAxis(ap=idx32[:, g:g + 1],
                                                         axis=0),
                    in_=zeros_sb[:],
                    in_offset=None,
                    bounds_check=C - 1,
                    oob_is_err=False)


def _get_nc():
    if "nc" not in _CACHE:
        _CACHE["nc"] = _build_nc()
    return _CACHE["nc"]


def _ef_cast_fp8(c, f8):
    """Cast [B,C,L] fp32 -> fp8 with per-(b,c) error-feedback along L, so the
    running sum of the quantized stream tracks the fp32 sum to ~1 ulp (keeps
    cond_pool = mean_L(c) accurate despite fp8 storage)."""
    c = np.asarray(c, np.float32)
    Bc, Cc, Lc = c.shape
    flat = c.reshape(Bc * Cc, Lc)
    out = np.empty((Bc * Cc, Lc), dtype=f8)
    carry = np.zeros((Bc * Cc,), np.float32)
    for l in range(Lc):
        v = flat[:, l] + carry
        q = v.astype(f8)
        out[:, l] = q
        carry = v - q.astype(np.float32)
    return out.reshape(Bc, Cc, Lc)


def _host_weight_maps(gw1, gb1, gw2, gb2, mw1, mb1, mw2, mb2):
    import ml_dtypes
    f = np.float32
    bf = ml_dtypes.bfloat16
    f8 = ml_dtypes.float8_e4m3
    w8 = f8 if W8 else bf
    # [C,H] -> [128, CG, H] (c-within-group on partitions, group-major free)
    w1t = np.asarray(gw1, f).T.reshape(CG, 128, H).transpose(1, 0, 2)
    m1t = np.asarray(mw1, f).T.reshape(CG, 128, H).transpose(1, 0, 2)
    w2t = np.asarray(gw2, f).T                                  # [H, C]
    wpk_f = np.zeros((128, NF), f)
    wpk_f[:, F_GB1] = np.asarray(gb1, f)
    wpk_f[:, F_GB2:F_GB2 + CG] = np.asarray(gb2, f).reshape(CG, 128).T
    wpk_f[:, F_MB1] = np.asarray(mb1, f)
    wpk_f[:, F_MB2:F_MB2 + 2 * CG] = np.asarray(mb2, f).reshape(2 * CG, 128).T
    wpk_f[:, F_ID:F_ID + 128] = np.eye(128, dtype=f)
    iota = np.arange(128, dtype=f)
    for g in range(CG):
        wpk_f[:, F_IOTA + g] = g * 128 + iota
    return {
        "w1p": np.ascontiguousarray(w1t.reshape(128, CG * H).astype(w8)),
        "w2p": np.ascontiguousarray(w2t.astype(w8)),
        "m1p": np.ascontiguousarray(m1t.reshape(128, CG * H).astype(bf)),
        "mw2p": np.ascontiguousarray(np.asarray(mw2, f).T),
        "wfp": np.ascontiguousarray(wpk_f),
    }


def _run(inputs, trace=False):
    import ml_dtypes
    from concourse.bass_utils import run_bass_kernel_spmd

    nc = _get_nc()
    bf = ml_dtypes.bfloat16
    f8 = ml_dtypes.float8_e4m3
    x = np.asarray(inputs["x"], np.float32).astype(bf)
    c8 = _ef_cast_fp8(inputs["c"], f8)
    wmap = _host_weight_maps(
        inputs["gw1"], inputs["gb1"], inputs["gw2"], inputs["gb2"],
        inputs["mw1"], inputs["mb1"], inputs["mw2"], inputs["mb2"])
    in_maps = [
        dict(wmap, x=np.ascontiguousarray(x[b]), c=np.ascontiguousarray(c8[b]))
        for b in range(B)
    ]
    res = run_bass_kernel_spmd(nc, in_maps, core_ids=list(range(B)), trace=trace)
    out = np.stack([np.asarray(res.results[b]["out"], np.float32) for b in range(B)],
                   axis=0)
    return out, res


def kernel(**inputs):
    out, _ = _run(inputs, trace=False)
    return out
